# revision 19
# baseline (speedup 1.0000x reference)
"""Trainium2 Bass kernel for ConditionalFeedForward (MoE routed FFN).

Problem: M=2048 tokens, D=1024, I=2048, E=8 experts, TOPK=2.
out[t, s] = FFN_{e}(x[t]) with e = expert_indices[t, s], where
FFN_e(x) = (silu(x @ w1_e.T) * (x @ w3_e.T)) @ w2_e.T  (w13 = [w1; w3]).

Strategy (expert parallelism, 8 experts -> 8 cores):
 - Host routes (token, slot) pairs to the core owning the selected expert,
   pads each core's token batch to a common capacity C, and transposes
   activations so features live on SBUF partitions and tokens on the free
   dim.  No device collectives needed: the "all-to-all" is a host gather
   and scatter around one SPMD kernel launch.
 - Per core: hT = w13_e @ x_eT via PE (fp32r, full rate at free dim >=256),
   g = silu(h1) * h3 on ACT+DVE, outT = w2_e.T-contract on PE, DMA out.
 - Weights stream through SBUF once (24 MB/core), pre-tiled on the host
   into the exact [128, k*128] lhsT layout the tensor engine wants.
"""

import os

import numpy as np

import concourse.bass as bass
import concourse.tile as tile
from concourse import bacc, mybir
from concourse.bass_utils import run_bass_kernel_spmd

M, D, I, E, TOPK = 2048, 1024, 2048, 8, 2
P = 128
KD = D // P            # 8   k-tiles over D (mm1 contraction)
NI2 = (2 * I) // P     # 32  n-tiles over 2I (mm1 output rows)
NPAIR = NI2 // 2       # 16  (x1, x3) pairs
KI = I // P            # 16  k-tiles over I (mm2 contraction)
ND = D // P            # 8   d-tiles over D (mm2 output rows)

F32 = mybir.dt.float32
F32R = mybir.dt.float32r

# exec time of the most recent launch (ns), populated when BASS_TRACE=1
LAST_EXEC_TIME_NS = None

_program_cache = {}


def _chunks_for(C):
    """Split C token-columns into matmul moving-dim chunks (<=512 each)."""
    n_ch = -(-C // 512)
    base = -(-C // (n_ch * 32)) * 32
    chunks = []
    off = 0
    while off < C:
        cn = min(base, C - off)
        chunks.append((off, cn))
        off += cn
    return tuple(chunks)


def _build_program(C):
    chunks = _chunks_for(C)
    nc = bacc.Bacc(
        "TRN2",
        target_bir_lowering=False,
        debug=False,
        enable_asserts=False,
        num_devices=E,
    )

    # x: partition-major fused layout [P, KD, C] (16KB/partition runs)
    # w13: (x1, x3) row-block PAIRS fused per partition row -> 8KB runs
    # w2: two d-blocks fused per row -> 16KB runs
    xT_d = nc.dram_tensor("xT", (P, KD, C), F32R, kind="ExternalInput").ap()
    w13_d = nc.dram_tensor(
        "w13t", (NPAIR, P, 2 * KD * P), F32R, kind="ExternalInput"
    ).ap()
    w2_d = nc.dram_tensor(
        "w2t", (ND // 2, P, 2 * KI * P), F32R, kind="ExternalInput"
    ).ap()
    out_d = nc.dram_tensor("outT", (ND, P, C), F32, kind="ExternalOutput").ap()

    with tile.TileContext(nc) as tc:
        with (
            tc.tile_pool(name="xg", bufs=1) as xg_pool,
            tc.tile_pool(name="wt", bufs=3) as w_pool,
            tc.tile_pool(name="tmp", bufs=4) as tmp_pool,
            tc.tile_pool(name="ps", bufs=8, space="PSUM") as ps_pool,
        ):
            W13F_BUFS = 4
            W13_AHEAD = W13F_BUFS - 1
            w13_buf = {}

            def issue_w13(pr):
                wAB = w_pool.tile(
                    [P, 2 * KD * P], F32R, tag="w13f", name="w13f", bufs=W13F_BUFS
                )
                nc.sync.dma_start(wAB[:], w13_d[pr])
                w13_buf[pr] = (wAB[:, : KD * P], wAB[:, KD * P :])

            W2F_BUFS = 3
            w2f_buf = {}

            def issue_w2f(j):
                wDD = w_pool.tile(
                    [P, 2 * KI * P], F32R, tag="w2f", name="w2f", bufs=W2F_BUFS
                )
                nc.sync.dma_start(wDD[:], w2_d[j])
                w2f_buf[j] = wDD

            # startup order: the first matmul needs only x[k=0] and pair-0 wA,
            # so those two small DMAs go first; everything else streams behind.
            xall = xg_pool.tile([P, KD * C], F32R, tag="x", name="x")
            x_tiles = [xall[:, k * C : (k + 1) * C] for k in range(KD)]
            nc.sync.dma_start(xall[:, :C], xT_d[:, 0, :])
            wA0 = w_pool.tile([P, KD * P], F32R, tag="w130", name="w130", bufs=1)
            nc.sync.dma_start(wA0[:], w13_d[0][:, : KD * P])
            wB0 = w_pool.tile([P, KD * P], F32R, tag="w131", name="w131", bufs=1)
            nc.sync.dma_start(wB0[:], w13_d[0][:, KD * P :])
            w13_buf[0] = (wA0, wB0)
            nc.sync.dma_start(
                xall[:, C:].rearrange("p (k c) -> p k c", c=C), xT_d[:, 1:, :]
            )
            for pr in range(1, 1 + W13_AHEAD):
                issue_w13(pr)

            g_tiles = [
                xg_pool.tile([P, C], F32R, tag=f"g{ki}", name=f"g{ki}")
                for ki in range(KI)
            ]

            # ---- mm1 + silu*gate: process (x1, x3) row-block pairs ----
            for pr in range(NPAIR):
                nxt = pr + 1 + W13_AHEAD
                if nxt < NPAIR:
                    issue_w13(nxt)
                elif nxt - NPAIR < min(W2F_BUFS, ND // 2):
                    issue_w2f(nxt - NPAIR)
                wA, wB = w13_buf.pop(pr)
                for c0, cn in chunks:
                    psA = ps_pool.tile([P, 512], F32, tag="ps", name="ps")[:, :cn]
                    psB = ps_pool.tile([P, 512], F32, tag="ps", name="ps")[:, :cn]
                    for k in range(KD):
                        nc.tensor.matmul(
                            psA,
                            wA[:, k * P : (k + 1) * P],
                            x_tiles[k][:, c0 : c0 + cn],
                            start=(k == 0),
                            stop=(k == KD - 1),
                        )
                    for k in range(KD):
                        nc.tensor.matmul(
                            psB,
                            wB[:, k * P : (k + 1) * P],
                            x_tiles[k][:, c0 : c0 + cn],
                            start=(k == 0),
                            stop=(k == KD - 1),
                        )
                    s = tmp_pool.tile([P, 512], F32, tag="s", name="s")[:, :cn]
                    nc.scalar.activation(s, psA, mybir.ActivationFunctionType.Silu)
                    nc.vector.tensor_mul(
                        out=g_tiles[pr][:, c0 : c0 + cn],
                        in0=s,
                        in1=psB,
                    )

            # ---- mm2: outT[d-block] = sum_ki w2T-tile @ g ----
            for d in range(ND):
                if d % 2 == 0:
                    j = d // 2
                    if j + W2F_BUFS < ND // 2:
                        issue_w2f(j + W2F_BUFS)
                wDD = w2f_buf[d // 2]
                wD = wDD[:, (d % 2) * KI * P : (d % 2 + 1) * KI * P]
                if d % 2 == 1:
                    w2f_buf.pop(d // 2)
                for c0, cn in chunks:
                    psO = ps_pool.tile([P, 512], F32, tag="ps", name="ps")[:, :cn]
                    for ki in range(KI):
                        nc.tensor.matmul(
                            psO,
                            wD[:, ki * P : (ki + 1) * P],
                            g_tiles[ki][:, c0 : c0 + cn],
                            start=(ki == 0),
                            stop=(ki == KI - 1),
                        )
                    ot = tmp_pool.tile([P, 512], F32, tag="o", name="o")[:, :cn]
                    nc.vector.tensor_copy(ot, psO)
                    nc.sync.dma_start(out_d[d][:, c0 : c0 + cn], ot)

    nc.compile()
    return nc


def _get_program(C):
    if C not in _program_cache:
        _program_cache[C] = _build_program(C)
    return _program_cache[C]


def _ensure_ntff_hook():
    """Provide antenv.axon_hooks if the image lacks it, so trace=True works."""
    import sys
    import types

    try:
        import antenv.axon_hooks  # noqa: F401

        return
    except ImportError:
        pass
    try:
        import antenv
        from trn_agent_boot.trn_boot import _ntff_profile_via_ctypes

        mod = types.ModuleType("antenv.axon_hooks")
        state = {"hook": None}
        mod.set_axon_ntff_profile_hook = lambda h: state.__setitem__("hook", h)
        mod.get_axon_ntff_profile_hook = lambda: state["hook"]
        sys.modules["antenv.axon_hooks"] = mod
        antenv.axon_hooks = mod
        mod.set_axon_ntff_profile_hook(
            _ntff_profile_via_ctypes("/opt/axon/libaxon_pjrt.so")
        )
    except Exception:
        pass


def kernel(x, w13, w2, expert_indices):
    global LAST_EXEC_TIME_NS
    x = np.asarray(x, dtype=np.float32)
    w13 = np.asarray(w13, dtype=np.float32)
    w2 = np.asarray(w2, dtype=np.float32)
    idx = np.asarray(expert_indices)
    out_idx_dtype = idx.dtype
    idx32 = idx.astype(np.int64)

    m, d_model = x.shape
    e, two_i, _ = w13.shape
    inter = w2.shape[2]
    topk = idx.shape[1]
    assert (m, d_model, e, two_i, inter, topk) == (M, D, E, 2 * I, I, TOPK)

    # ---- host routing: unique (token, expert) work items per expert ----
    # A token picking the same expert in both slots computes the FFN once;
    # the result is scattered to every matching slot.
    tok_unique = [
        np.unique(np.concatenate([np.nonzero(idx32[:, s] == ei)[0] for s in range(topk)]))
        for ei in range(E)
    ]
    max_cnt = max(len(u) for u in tok_unique)
    C = max(256, int(-(-max_cnt // 8) * 8))

    nc = _get_program(C)

    in_maps = []
    for ei in range(E):
        tok_ids = tok_unique[ei]
        cnt = len(tok_ids)

        xg = np.zeros((C, D), dtype=np.float32)
        xg[:cnt] = x[tok_ids]
        xT = np.ascontiguousarray(
            xg.T.reshape(KD, P, C).transpose(1, 0, 2)
        )                                            # [p, k, c]

        A4 = w13[ei].reshape(NI2, P, KD, P)          # [n, c, k, p]
        w13t = A4.transpose(0, 3, 2, 1).reshape(NI2, P, KD * P)
        w13p = np.ascontiguousarray(
            np.concatenate([w13t[:NPAIR], w13t[NPAIR:]], axis=2)
        )                                            # [pair, p, 2*KD*P]
        B4 = w2[ei].reshape(ND, P, KI, P)            # [d, c, ki, p]
        w2t = B4.transpose(0, 3, 2, 1).reshape(ND, P, KI * P)
        w2p = np.ascontiguousarray(
            w2t.reshape(ND // 2, 2, P, KI * P).transpose(0, 2, 1, 3).reshape(
                ND // 2, P, 2 * KI * P
            )
        )                                            # [dpair, p, 2*KI*P]

        in_maps.append({"xT": xT, "w13t": w13p, "w2t": w2p})

    trace = bool(os.environ.get("BASS_TRACE"))
    if trace:
        _ensure_ntff_hook()
    res = run_bass_kernel_spmd(nc, in_maps, core_ids=list(range(E)), trace=trace)
    LAST_EXEC_TIME_NS = res.exec_time_ns

    # ---- host scatter: copy each expert's outputs to all matching slots ----
    out = np.empty((M, topk, D), dtype=np.float32)
    for ei in range(E):
        outT = res.results[ei]["outT"].reshape(D, C)
        oe = outT[:, : len(tok_unique[ei])].T        # [cnt, D]
        for s in range(topk):
            sel = np.nonzero(idx32[:, s] == ei)[0]
            out[sel, s] = oe[np.searchsorted(tok_unique[ei], sel)]

    del out_idx_dtype
    return out
